# revision 17
# baseline (speedup 1.0000x reference)
"""Trainium2 Bass kernel for nn_MultiHeadAttention_8074538516581.

Sharding: 8 cores = batch(4) x head-group(2 groups of 6 heads).
Each core computes, for its (b, g):
  qkv slice projection (bf16 matmuls, fp32 psum accum, struct-embed term
  folded in as a rank-4 matmul O @ (SE @ W^T)), per-head attention with the
  reference's exact semantics (q/k rounded to bf16, fixed-shift-free softmax
  -- the row-max subtraction cancels in the normalization, the [-30,30] logit
  clip and the 1e5/1e-10 guards are provably inactive here), and the partial
  output projection over its 384 head-dims.
Host sums the two head-group partials per batch and adds b_out.

Token permutation: queries with (t % 64) % 3 == 0 are zeroed by the
reference's load mask, making their attention output mean(v) per head.
Tokens are permuted live-first so the 672 live queries are contiguous:
scores/exp/pv run only on live columns; the 352 masked columns get the
per-head mean(v) via one N=1 matmul + broadcast.
"""
import numpy as np
import ml_dtypes

import concourse.bass as bass
import concourse.mybir as mybir
import concourse.tile as tile
from concourse import bacc
from concourse.bass import ts
from concourse.bass_utils import run_bass_kernel_spmd

B, T, E = 4, 1024, 768
H, D = 12, 64
HG = 6                  # heads per group
GD = HG * D             # 384 head-dims per group
BLOCK_M = 64
LIVE = 672              # tokens with (t % BLOCK_M) % 3 != 0
MASK = T - LIVE         # 352
SCALE = 1.0 / 8.0       # 1/sqrt(64)

BF16 = mybir.dt.bfloat16
F32 = mybir.dt.float32

_perm = None
_nc = None


def _perm_live_first():
    t = np.arange(T)
    m = (t % BLOCK_M) % 3 == 0
    return np.concatenate([t[~m], t[m]])


def _build_bass(debug=False, repeat=1):
    nc = bacc.Bacc()
    xT_d = nc.dram_tensor("xT", [E, T], BF16, kind="ExternalInput")
    wT_d = nc.dram_tensor("wT", [E, 3 * GD], BF16, kind="ExternalInput")
    ot_d = nc.dram_tensor("ot", [4, T], BF16, kind="ExternalInput")
    m2_d = nc.dram_tensor("m2", [4, 3 * GD], BF16, kind="ExternalInput")
    woT_d = nc.dram_tensor("woT", [GD, E], BF16, kind="ExternalInput")
    out_d = nc.dram_tensor("out", [T, E], F32, kind="ExternalOutput")

    dbg_p = None
    if debug:
        dbg_p = nc.dram_tensor("dbg_p", [128, 8, LIVE], BF16, kind="ExternalOutput")
        dbg_s = nc.dram_tensor("dbg_s", [128, 8, LIVE], F32, kind="ExternalOutput")

    from contextlib import ExitStack
    with tile.TileContext(nc) as tc, ExitStack() as rep_ctx:
        if repeat > 1:
            rep_ctx.enter_context(tc.For_i(0, repeat, 1))
        with tc.tile_pool(name="singles", bufs=1) as singles:
            xT_sb = singles.tile([128, 6, T], BF16)
            wT_sb = singles.tile([128, 6, 3 * GD], BF16)
            woT_sb = singles.tile([128, 3, E], BF16)
            ot_sb = singles.tile([4, T], BF16)
            m2_sb = singles.tile([4, 3 * GD], BF16)
            ones_p = singles.tile([128, MASK], BF16)
            qT_sb = singles.tile([128, 3, T], BF16)   # cols LIVE: garbage, never read
            kT_sb = singles.tile([128, 3, T], BF16)
            v_sb = singles.tile([128, 8, HG * (D + 1)], BF16)  # per-head v | ones col
            attnT_sb = singles.tile([128, 3, T], BF16)

            nc.sync.dma_start(out=xT_sb, in_=xT_d[:, :].rearrange("(c p) t -> p c t", p=128))
            nc.sync.dma_start(out=wT_sb, in_=wT_d[:, :].rearrange("(c p) t -> p c t", p=128))
            nc.sync.dma_start(out=woT_sb, in_=woT_d[:, :].rearrange("(c p) t -> p c t", p=128))
            nc.sync.dma_start(out=ot_sb, in_=ot_d[:, :])
            nc.sync.dma_start(out=m2_sb, in_=m2_d[:, :])
            nc.vector.memset(ones_p, 1.0)
            v_ones = v_sb[:, :, :].rearrange("p a (h e) -> p a h e", e=D + 1)[:, :, :, D:D + 1]
            nc.vector.memset(v_ones, 1.0)
            # q column LIVE is pinned to 0 so exp gives p'=1 there: the pv
            # matmul's column LIVE-512 then lands [sum(v) | 1024] = the
            # masked-query numerator and denominator, with a single
            # start=True writer chain per PSUM bank.
            nc.vector.memset(qT_sb[:, :, LIVE:LIVE + 1], 0.0)

            # ---- Phase 1: v projection (natural layout, feeds all heads)
            with tc.tile_pool(name="v_ps", bufs=2, space="PSUM") as v_pool:
                for tt in range(8):
                    ps = v_pool.tile([128, GD], F32, tag="vps")
                    for ek in range(6):
                        nc.tensor.matmul(ps,
                                         xT_sb[:, ek, ts(tt, 128)],
                                         wT_sb[:, ek, 2 * GD:3 * GD],
                                         start=(ek == 0), stop=False)
                    nc.tensor.matmul(ps, ot_sb[:, ts(tt, 128)],
                                     m2_sb[:, 2 * GD:3 * GD], start=False, stop=True)
                    dst = v_sb[:, tt, :].rearrange("p (h e) -> p h e", e=D + 1)[:, :, 0:D]
                    src = ps[:, :].rearrange("p (h d) -> p h d", d=D)
                    nc.scalar.copy(dst, src)

            # ---- Phase 2: per head-pair: project q,k chunk then attend both
            # heads. Keeps PE dense (projection of pair c+1 overlaps the
            # ACT-bound softmax of pair c) so HAM stays warm.
            with tc.tile_pool(name="qk_ps", bufs=1, space="PSUM") as qk_pool, \
                 tc.tile_pool(name="sT_ps", bufs=2, space="PSUM") as sT_pool, \
                 tc.tile_pool(name="acc_ps", bufs=1, space="PSUM") as acc_pool, \
                 tc.tile_pool(name="acc2_ps", bufs=1, space="PSUM") as acc2_pool, \
                 tc.tile_pool(name="pp", bufs=3) as pp_pool, \
                 tc.tile_pool(name="sm", bufs=3) as sm_pool:
                for c in range(3):
                    for mt in (c, c + 3):    # q chunk then k chunk
                        ps = qk_pool.tile([128, T], F32, tag="qkps")
                        isq = mt < 3
                        slices = ((0, 512), (512, LIVE)) if isq else ((0, 512), (512, T))
                        for ek in range(6):
                            for s0, s1 in slices:
                                nc.tensor.matmul(ps[:, s0:s1],
                                                 wT_sb[:, ek, ts(mt, 128)],
                                                 xT_sb[:, ek, s0:s1],
                                                 start=(ek == 0), stop=False)
                        for s0, s1 in slices:
                            nc.tensor.matmul(ps[:, s0:s1],
                                             m2_sb[:, ts(mt, 128)],
                                             ot_sb[:, s0:s1],
                                             start=False, stop=True)
                        if isq:
                            nc.vector.tensor_copy(qT_sb[:, mt, 0:LIVE], ps[:, 0:LIVE])
                        else:
                            nc.vector.tensor_copy(kT_sb[:, mt - 3, :], ps[:, :])

                    for h in (2 * c, 2 * c + 1):
                        po = (h % 2) * 64
                        qh = qT_sb[po:po + 64, c, :]
                        kh = kT_sb[po:po + 64, c, :]
                        acc1 = acc_pool.tile([65, 512], F32, tag="acc1")
                        acc2 = acc2_pool.tile([65, 512], F32, tag="acc2")
                        for kt in range(8):
                            # [0:512) in bank 0, [512:673) in bank 1, aligned
                            sT = sT_pool.tile([128, T], F32, tag="sT")
                            pp = pp_pool.tile([128, LIVE + 1], BF16, tag="pp")
                            nc.tensor.matmul(sT[:, 0:512], kh[:, ts(kt, 128)],
                                             qh[:, 0:512], start=True, stop=True)
                            nc.tensor.matmul(sT[:, 512:LIVE + 1], kh[:, ts(kt, 128)],
                                             qh[:, 512:LIVE + 1], start=True, stop=True)
                            nc.scalar.activation(pp[:, 0:LIVE + 1], sT[:, 0:LIVE + 1],
                                                 mybir.ActivationFunctionType.Exp,
                                                 scale=SCALE)
                            if debug and h == 0:
                                nc.sync.dma_start(out=dbg_p[:, kt, :], in_=pp[:, 0:LIVE])
                                sc = sm_pool.tile([128, LIVE], F32, tag="dbgsc")
                                nc.vector.tensor_copy(sc, sT[:, 0:LIVE])
                                nc.sync.dma_start(out=dbg_s[:, kt, :], in_=sc)
                            vh = v_sb[:, kt, h * (D + 1):(h + 1) * (D + 1)]
                            nc.tensor.matmul(acc1[:, :], vh, pp[:, 0:512],
                                             start=(kt == 0), stop=(kt == 7))
                            nc.tensor.matmul(acc2[:, 0:LIVE - 512 + 1], vh,
                                             pp[:, 512:LIVE + 1],
                                             start=(kt == 0), stop=(kt == 7))
                        # epilogue: normalize by denominators (row 64 of acc)
                        rd1 = sm_pool.tile([1, 512], F32, tag="rd1")
                        rd2 = sm_pool.tile([1, LIVE - 512], F32, tag="rd2")
                        rd3 = sm_pool.tile([1, 1], F32, tag="rd3")
                        nc.vector.reciprocal(rd1, acc1[64:65, :])
                        nc.vector.reciprocal(rd2, acc2[64:65, 0:LIVE - 512])
                        nc.vector.reciprocal(rd3, acc2[64:65, LIVE - 512:LIVE - 512 + 1])
                        rb1 = sm_pool.tile([64, 512], F32, tag="rb1")
                        rb2 = sm_pool.tile([64, LIVE - 512], F32, tag="rb2")
                        rb3 = sm_pool.tile([64, 1], F32, tag="rb3")
                        for rb, rd in ((rb1, rd1), (rb2, rd2), (rb3, rd3)):
                            nc.gpsimd.partition_broadcast(rb, rd[0:1, :])
                        ah = attnT_sb[po:po + 64, c, :]
                        nc.vector.tensor_mul(ah[:, 0:512], acc1[0:64, :], rb1)
                        nc.vector.tensor_mul(ah[:, 512:LIVE], acc2[0:64, 0:LIVE - 512], rb2)
                        mv = sm_pool.tile([64, 1], F32, tag="mv")
                        nc.vector.tensor_scalar_mul(mv, acc2[0:64, LIVE - 512:LIVE - 512 + 1], rb3)
                        nc.vector.tensor_scalar_mul(ah[:, LIVE:T], ones_p[0:64, :], mv)

            if debug:
                for nm, t, sh in (("dbg_q", qT_sb, [128, 3, T]),
                                  ("dbg_k", kT_sb, [128, 3, T]),
                                  ("dbg_v", v_sb, [128, 8, HG * (D + 1)]),
                                  ("dbg_a", attnT_sb, [128, 3, T])):
                    dd = nc.dram_tensor(nm, sh, BF16, kind="ExternalOutput")
                    nc.sync.dma_start(out=dd[:, :, :], in_=t[:, :, :])

            # ---- Phase 3: output projection (partial over this group's dims)
            with tc.tile_pool(name="o_ps", bufs=3, space="PSUM") as o_pool, \
                 tc.tile_pool(name="ob", bufs=3) as ob_pool:
                for tt in range(8):
                    ps = o_pool.tile([128, E], F32, tag="ops")
                    for s0, s1 in ((0, 512), (512, E)):
                        for c3 in range(3):
                            nc.tensor.matmul(ps[:, s0:s1],
                                             attnT_sb[:, c3, ts(tt, 128)],
                                             woT_sb[:, c3, s0:s1],
                                             start=(c3 == 0), stop=(c3 == 2))
                    ob = ob_pool.tile([128, E], F32, tag="ob")
                    nc.vector.tensor_copy(ob, ps)
                    nc.sync.dma_start(out=out_d[ts(tt, 128), :], in_=ob)

    nc.finalize()
    return nc


def _get_bass():
    global _nc
    if _nc is None:
        _nc = _build_bass()
    return _nc


def kernel(x, idx, struct_embed, w_qkv, w_out, b_out):
    global _perm
    if _perm is None:
        _perm = _perm_live_first()
    perm = _perm

    x = np.asarray(x, dtype=np.float32)
    idx = np.asarray(idx)
    struct_embed = np.asarray(struct_embed, dtype=np.float32)
    w_qkv = np.asarray(w_qkv, dtype=np.float32)
    w_out = np.asarray(w_out, dtype=np.float32)
    b_out = np.asarray(b_out, dtype=np.float32)

    sid = ((idx == 1) * 1 + (idx == 2) * 2 + (idx == 3) * 3).astype(np.int64)  # [B,T]
    oh = (sid[:, :, None] == np.arange(4)[None, None, :]).astype(np.float32)  # [B,T,4]

    bf = ml_dtypes.bfloat16
    in_maps = []
    for core in range(8):
        b, g = core // 2, core % 2
        wg = np.concatenate([w_qkv[g * GD:(g + 1) * GD],
                             w_qkv[E + g * GD:E + (g + 1) * GD],
                             w_qkv[2 * E + g * GD:2 * E + (g + 1) * GD]], axis=0)  # [3GD, E]
        in_maps.append({
            "xT": np.ascontiguousarray(x[b].T[:, perm]).astype(bf),
            "wT": np.ascontiguousarray(wg.T).astype(bf),
            "ot": np.ascontiguousarray(oh[b].T[:, perm]).astype(bf),
            "m2": (struct_embed @ wg.T).astype(bf),
            "woT": np.ascontiguousarray(w_out[:, g * GD:(g + 1) * GD].T).astype(bf),
        })

    res = run_bass_kernel_spmd(_get_bass(), in_maps, core_ids=list(range(8)))

    inv = np.empty(T, dtype=np.int64)
    inv[perm] = np.arange(T)
    out = np.empty((B, T, E), dtype=np.float32)
    for b in range(B):
        acc = res.results[2 * b]["out"] + res.results[2 * b + 1]["out"]
        out[b] = acc[inv] + b_out[None, :]
    return out


# revision 19
# speedup vs baseline: 1.2867x; 1.2867x over previous
"""Trainium2 Bass kernel for nn_MultiHeadAttention_8074538516581.

Sharding: 8 cores = batch(4) x head-group(2 groups of 6 heads).
Each core computes, for its (b, g):
  qkv slice projection (bf16 matmuls, fp32 psum accum, struct-embed term
  folded in as a rank-4 matmul O @ (SE @ W^T)), per-head attention with the
  reference's exact semantics (q/k rounded to bf16, fixed-shift-free softmax
  -- the row-max subtraction cancels in the normalization, the [-30,30] logit
  clip and the 1e5/1e-10 guards are provably inactive here), and the partial
  output projection over its 384 head-dims.
Host sums the two head-group partials per batch and adds b_out.

Token permutation: queries with (t % 64) % 3 == 0 are zeroed by the
reference's load mask, making their attention output mean(v) per head.
Tokens are permuted live-first so the 672 live queries are contiguous:
scores/exp/pv run only on live columns; the 352 masked columns get the
per-head mean(v) via one N=1 matmul + broadcast.
"""
import numpy as np
import ml_dtypes

import concourse.bass as bass
import concourse.mybir as mybir
import concourse.tile as tile
from concourse import bacc
from concourse.bass import ts
from concourse.bass_utils import run_bass_kernel_spmd

B, T, E = 4, 1024, 768
H, D = 12, 64
HG = 6                  # heads per group
GD = HG * D             # 384 head-dims per group
BLOCK_M = 64
LIVE = 672              # tokens with (t % BLOCK_M) % 3 != 0
MASK = T - LIVE         # 352
SCALE = 1.0 / 8.0       # 1/sqrt(64)

BF16 = mybir.dt.bfloat16
F32 = mybir.dt.float32

_perm = None
_nc = None


def _perm_live_first():
    t = np.arange(T)
    m = (t % BLOCK_M) % 3 == 0
    return np.concatenate([t[~m], t[m]])


def _build_bass(debug=False, repeat=1):
    nc = bacc.Bacc()
    xT_d = nc.dram_tensor("xT", [E, T], BF16, kind="ExternalInput")
    wT_d = nc.dram_tensor("wT", [E, 3 * GD], BF16, kind="ExternalInput")
    ot_d = nc.dram_tensor("ot", [4, T], BF16, kind="ExternalInput")
    m2_d = nc.dram_tensor("m2", [4, 3 * GD], BF16, kind="ExternalInput")
    woT_d = nc.dram_tensor("woT", [GD, E], BF16, kind="ExternalInput")
    out_d = nc.dram_tensor("out", [T, E], F32, kind="ExternalOutput")

    dbg_p = None
    if debug:
        dbg_p = nc.dram_tensor("dbg_p", [128, 8, LIVE], BF16, kind="ExternalOutput")
        dbg_s = nc.dram_tensor("dbg_s", [128, 8, LIVE], F32, kind="ExternalOutput")

    from contextlib import ExitStack
    with tile.TileContext(nc) as tc, ExitStack() as rep_ctx:
        if repeat > 1:
            rep_ctx.enter_context(tc.For_i(0, repeat, 1))
        with tc.tile_pool(name="singles", bufs=1) as singles:
            xT_sb = singles.tile([128, 6, T], BF16)
            wT_sb = singles.tile([128, 6, 3 * GD], BF16)
            woT_sb = singles.tile([128, 3, E], BF16)
            ot_sb = singles.tile([4, T], BF16)
            m2_sb = singles.tile([4, 3 * GD], BF16)
            ones_p = singles.tile([128, MASK], BF16)
            qT_sb = singles.tile([128, 3, T], BF16)   # cols LIVE: garbage, never read
            kT_sb = singles.tile([128, 3, T], BF16)
            v_sb = singles.tile([128, 8, HG * (D + 1)], BF16)  # per-head v | ones col
            attnT_sb = singles.tile([128, 3, T], BF16)

            nc.sync.dma_start(out=xT_sb, in_=xT_d[:, :].rearrange("(c p) t -> p c t", p=128))
            nc.sync.dma_start(out=wT_sb, in_=wT_d[:, :].rearrange("(c p) t -> p c t", p=128))
            nc.sync.dma_start(out=woT_sb, in_=woT_d[:, :].rearrange("(c p) t -> p c t", p=128))
            nc.sync.dma_start(out=ot_sb, in_=ot_d[:, :])
            nc.sync.dma_start(out=m2_sb, in_=m2_d[:, :])
            nc.vector.memset(ones_p, 1.0)
            v_ones = v_sb[:, :, :].rearrange("p a (h e) -> p a h e", e=D + 1)[:, :, :, D:D + 1]
            nc.vector.memset(v_ones, 1.0)
            # q column LIVE is pinned to 0 so exp gives p'=1 there: the pv
            # matmul's column LIVE-512 then lands [sum(v) | 1024] = the
            # masked-query numerator and denominator, with a single
            # start=True writer chain per PSUM bank.
            nc.vector.memset(qT_sb[:, :, LIVE:LIVE + 1], 0.0)

            # ---- Phase 1: v projection (natural layout, feeds all heads)
            with tc.tile_pool(name="v_ps", bufs=2, space="PSUM") as v_pool:
                for tt in range(8):
                    ps = v_pool.tile([128, GD], F32, tag="vps")
                    for ek in range(6):
                        nc.tensor.matmul(ps,
                                         xT_sb[:, ek, ts(tt, 128)],
                                         wT_sb[:, ek, 2 * GD:3 * GD],
                                         start=(ek == 0), stop=False)
                    nc.tensor.matmul(ps, ot_sb[:, ts(tt, 128)],
                                     m2_sb[:, 2 * GD:3 * GD], start=False, stop=True)
                    dst = v_sb[:, tt, :].rearrange("p (h e) -> p h e", e=D + 1)[:, :, 0:D]
                    src = ps[:, :].rearrange("p (h d) -> p h d", d=D)
                    nc.scalar.copy(dst, src)

            # ---- Phase 2: per head-pair: project q,k chunk then attend both
            # heads. Keeps PE dense (projection of pair c+1 overlaps the
            # ACT-bound softmax of pair c) so HAM stays warm.
            with tc.tile_pool(name="qk_ps", bufs=1, space="PSUM") as qk_pool, \
                 tc.tile_pool(name="sT_ps", bufs=2, space="PSUM") as sT_pool, \
                 tc.tile_pool(name="acc_ps", bufs=1, space="PSUM") as acc_pool, \
                 tc.tile_pool(name="acc2_ps", bufs=1, space="PSUM") as acc2_pool, \
                 tc.tile_pool(name="pp", bufs=3) as pp_pool, \
                 tc.tile_pool(name="sm", bufs=3) as sm_pool, \
                 tc.tile_pool(name="dscr", bufs=3, space="DRAM") as dr_pool:
                for c in range(3):
                    for mt in (c, c + 3):    # q chunk then k chunk
                        ps = qk_pool.tile([128, T], F32, tag="qkps")
                        isq = mt < 3
                        slices = ((0, 512), (512, LIVE)) if isq else ((0, 512), (512, T))
                        for ek in range(6):
                            for s0, s1 in slices:
                                nc.tensor.matmul(ps[:, s0:s1],
                                                 wT_sb[:, ek, ts(mt, 128)],
                                                 xT_sb[:, ek, s0:s1],
                                                 start=(ek == 0), stop=False)
                        for s0, s1 in slices:
                            nc.tensor.matmul(ps[:, s0:s1],
                                             m2_sb[:, ts(mt, 128)],
                                             ot_sb[:, s0:s1],
                                             start=False, stop=True)
                        if isq:
                            nc.vector.tensor_copy(qT_sb[:, mt, 0:LIVE], ps[:, 0:LIVE])
                        else:
                            nc.vector.tensor_copy(kT_sb[:, mt - 3, :], ps[:, :])

                    for h in (2 * c, 2 * c + 1):
                        po = (h % 2) * 64
                        qh = qT_sb[po:po + 64, c, :]
                        kh = kT_sb[po:po + 64, c, :]
                        acc1 = acc_pool.tile([65, 512], F32, tag="acc1")
                        acc2 = acc2_pool.tile([65, 512], F32, tag="acc2")
                        for kt in range(8):
                            # [0:512) in bank 0, [512:673) in bank 1, aligned
                            sT = sT_pool.tile([128, T], F32, tag="sT")
                            pp = pp_pool.tile([128, LIVE + 1], BF16, tag="pp")
                            nc.tensor.matmul(sT[:, 0:512], kh[:, ts(kt, 128)],
                                             qh[:, 0:512], start=True, stop=True)
                            nc.tensor.matmul(sT[:, 512:LIVE + 1], kh[:, ts(kt, 128)],
                                             qh[:, 512:LIVE + 1], start=True, stop=True)
                            nc.scalar.activation(pp[:, 0:LIVE + 1], sT[:, 0:LIVE + 1],
                                                 mybir.ActivationFunctionType.Exp,
                                                 scale=SCALE)
                            if debug and h == 0:
                                nc.sync.dma_start(out=dbg_p[:, kt, :], in_=pp[:, 0:LIVE])
                                sc = sm_pool.tile([128, LIVE], F32, tag="dbgsc")
                                nc.vector.tensor_copy(sc, sT[:, 0:LIVE])
                                nc.sync.dma_start(out=dbg_s[:, kt, :], in_=sc)
                            vh = v_sb[:, kt, h * (D + 1):(h + 1) * (D + 1)]
                            nc.tensor.matmul(acc1[:, :], vh, pp[:, 0:512],
                                             start=(kt == 0), stop=(kt == 7))
                            nc.tensor.matmul(acc2[:, 0:LIVE - 512 + 1], vh,
                                             pp[:, 512:LIVE + 1],
                                             start=(kt == 0), stop=(kt == 7))
                        # epilogue: normalize by denominators (row 64 of acc)
                        rd1 = sm_pool.tile([1, 512], F32, tag="rd1")
                        rd2 = sm_pool.tile([1, LIVE - 512], F32, tag="rd2")
                        rd3 = sm_pool.tile([1, 1], F32, tag="rd3")
                        nc.vector.reciprocal(rd1, acc1[64:65, :])
                        nc.vector.reciprocal(rd2, acc2[64:65, 0:LIVE - 512])
                        nc.vector.reciprocal(rd3, acc2[64:65, LIVE - 512:LIVE - 512 + 1])
                        rb1 = sm_pool.tile([64, 512], F32, tag="rb1")
                        rb2 = sm_pool.tile([64, LIVE - 512], F32, tag="rb2")
                        rb3 = sm_pool.tile([64, 1], F32, tag="rb3")
                        for rb, rd, w in ((rb1, rd1, 512), (rb2, rd2, LIVE - 512),
                                          (rb3, rd3, 1)):
                            # partition-broadcast via DRAM roundtrip (DMA can
                            # replicate from linear memory; SBUF-source
                            # zero-stride partition APs are not allowed)
                            dscr = dr_pool.tile([1, w], F32, tag=f"d{w}")
                            nc.sync.dma_start(out=dscr, in_=rd[0:1, :])
                            src = dscr[0:1, :]
                            bc = bass.AP(tensor=src.tensor, offset=src.offset,
                                         ap=[[0, 64]] + [list(a) for a in src.ap[1:]])
                            nc.sync.dma_start(out=rb, in_=bc)
                        ah = attnT_sb[po:po + 64, c, :]
                        nc.vector.tensor_mul(ah[:, 0:512], acc1[0:64, :], rb1)
                        nc.vector.tensor_mul(ah[:, 512:LIVE], acc2[0:64, 0:LIVE - 512], rb2)
                        mv = sm_pool.tile([64, 1], F32, tag="mv")
                        nc.vector.tensor_scalar_mul(mv, acc2[0:64, LIVE - 512:LIVE - 512 + 1], rb3)
                        nc.vector.tensor_scalar_mul(ah[:, LIVE:T], ones_p[0:64, :], mv)

            if debug:
                for nm, t, sh in (("dbg_q", qT_sb, [128, 3, T]),
                                  ("dbg_k", kT_sb, [128, 3, T]),
                                  ("dbg_v", v_sb, [128, 8, HG * (D + 1)]),
                                  ("dbg_a", attnT_sb, [128, 3, T])):
                    dd = nc.dram_tensor(nm, sh, BF16, kind="ExternalOutput")
                    nc.sync.dma_start(out=dd[:, :, :], in_=t[:, :, :])

            # ---- Phase 3: output projection (partial over this group's dims)
            with tc.tile_pool(name="o_ps", bufs=3, space="PSUM") as o_pool, \
                 tc.tile_pool(name="ob", bufs=3) as ob_pool:
                for tt in range(8):
                    ps = o_pool.tile([128, E], F32, tag="ops")
                    for s0, s1 in ((0, 512), (512, E)):
                        for c3 in range(3):
                            nc.tensor.matmul(ps[:, s0:s1],
                                             attnT_sb[:, c3, ts(tt, 128)],
                                             woT_sb[:, c3, s0:s1],
                                             start=(c3 == 0), stop=(c3 == 2))
                    ob = ob_pool.tile([128, E], F32, tag="ob")
                    nc.vector.tensor_copy(ob, ps)
                    nc.sync.dma_start(out=out_d[ts(tt, 128), :], in_=ob)

    nc.finalize()
    return nc


def _get_bass():
    global _nc
    if _nc is None:
        _nc = _build_bass()
    return _nc


def kernel(x, idx, struct_embed, w_qkv, w_out, b_out):
    global _perm
    if _perm is None:
        _perm = _perm_live_first()
    perm = _perm

    x = np.asarray(x, dtype=np.float32)
    idx = np.asarray(idx)
    struct_embed = np.asarray(struct_embed, dtype=np.float32)
    w_qkv = np.asarray(w_qkv, dtype=np.float32)
    w_out = np.asarray(w_out, dtype=np.float32)
    b_out = np.asarray(b_out, dtype=np.float32)

    sid = ((idx == 1) * 1 + (idx == 2) * 2 + (idx == 3) * 3).astype(np.int64)  # [B,T]
    oh = (sid[:, :, None] == np.arange(4)[None, None, :]).astype(np.float32)  # [B,T,4]

    bf = ml_dtypes.bfloat16
    in_maps = []
    for core in range(8):
        b, g = core // 2, core % 2
        wg = np.concatenate([w_qkv[g * GD:(g + 1) * GD],
                             w_qkv[E + g * GD:E + (g + 1) * GD],
                             w_qkv[2 * E + g * GD:2 * E + (g + 1) * GD]], axis=0)  # [3GD, E]
        in_maps.append({
            "xT": np.ascontiguousarray(x[b].T[:, perm]).astype(bf),
            "wT": np.ascontiguousarray(wg.T).astype(bf),
            "ot": np.ascontiguousarray(oh[b].T[:, perm]).astype(bf),
            "m2": (struct_embed @ wg.T).astype(bf),
            "woT": np.ascontiguousarray(w_out[:, g * GD:(g + 1) * GD].T).astype(bf),
        })

    res = run_bass_kernel_spmd(_get_bass(), in_maps, core_ids=list(range(8)))

    inv = np.empty(T, dtype=np.int64)
    inv[perm] = np.arange(T)
    out = np.empty((B, T, E), dtype=np.float32)
    for b in range(B):
        acc = res.results[2 * b]["out"] + res.results[2 * b + 1]["out"]
        out[b] = acc[inv] + b_out[None, :]
    return out


# revision 21
# speedup vs baseline: 3.7227x; 2.8931x over previous
"""Trainium2 Bass kernel for nn_MultiHeadAttention_8074538516581.

Sharding: 8 cores = batch(4) x head-group(2 groups of 6 heads).
Each core computes, for its (b, g):
  qkv slice projection (bf16 matmuls, fp32 psum accum, struct-embed term
  folded in as a rank-4 matmul O @ (SE @ W^T)), per-head attention with the
  reference's exact semantics (q/k rounded to bf16, fixed-shift-free softmax
  -- the row-max subtraction cancels in the normalization, the [-30,30] logit
  clip and the 1e5/1e-10 guards are provably inactive here), and the partial
  output projection over its 384 head-dims.
Host sums the two head-group partials per batch and adds b_out.

Token permutation: queries with (t % 64) % 3 == 0 are zeroed by the
reference's load mask, making their attention output mean(v) per head.
Tokens are permuted live-first so the 672 live queries are contiguous:
scores/exp/pv run only on live columns; the 352 masked columns get the
per-head mean(v) via one N=1 matmul + broadcast.
"""
import numpy as np
import ml_dtypes

import concourse.bass as bass
import concourse.mybir as mybir
import concourse.tile as tile
from concourse import bacc
from concourse.bass import ts
from concourse.bass_utils import run_bass_kernel_spmd

B, T, E = 4, 1024, 768
H, D = 12, 64
HG = 6                  # heads per group
GD = HG * D             # 384 head-dims per group
BLOCK_M = 64
LIVE = 672              # tokens with (t % BLOCK_M) % 3 != 0
MASK = T - LIVE         # 352
SCALE = 1.0 / 8.0       # 1/sqrt(64)

BF16 = mybir.dt.bfloat16
F32 = mybir.dt.float32

_perm = None
_nc = None


def _perm_live_first():
    t = np.arange(T)
    m = (t % BLOCK_M) % 3 == 0
    return np.concatenate([t[~m], t[m]])


def _build_bass(debug=False, repeat=1, upto="full"):
    nc = bacc.Bacc()
    xT_d = nc.dram_tensor("xT", [E, T], BF16, kind="ExternalInput")
    wT_d = nc.dram_tensor("wT", [E, 3 * GD], BF16, kind="ExternalInput")
    ot_d = nc.dram_tensor("ot", [4, T], BF16, kind="ExternalInput")
    m2_d = nc.dram_tensor("m2", [4, 3 * GD], BF16, kind="ExternalInput")
    woT_d = nc.dram_tensor("woT", [GD, E], BF16, kind="ExternalInput")
    out_d = nc.dram_tensor("out", [T, E], F32, kind="ExternalOutput")

    dbg_p = None
    if debug:
        dbg_p = nc.dram_tensor("dbg_p", [128, 8, LIVE], BF16, kind="ExternalOutput")
        dbg_s = nc.dram_tensor("dbg_s", [128, 8, LIVE], F32, kind="ExternalOutput")

    from contextlib import ExitStack
    with tile.TileContext(nc) as tc, ExitStack() as rep_ctx:
        if repeat > 1:
            rep_ctx.enter_context(tc.For_i(0, repeat, 1))
        with tc.tile_pool(name="singles", bufs=1) as singles:
            xT_sb = singles.tile([128, 6, T], BF16)
            wT_sb = singles.tile([128, 6, 3 * GD], BF16)
            woT_sb = singles.tile([128, 3, E], BF16)
            ot_sb = singles.tile([4, T], BF16)
            m2_sb = singles.tile([4, 3 * GD], BF16)
            ones_p = singles.tile([128, MASK], BF16)
            qT_sb = singles.tile([128, 3, T], BF16)   # cols LIVE: garbage, never read
            kT_sb = singles.tile([128, 3, T], BF16)
            v_sb = singles.tile([128, 8, HG * (D + 1)], BF16)  # per-head v | ones col
            attnT_sb = singles.tile([128, 3, T], BF16)

            nc.sync.dma_start(out=xT_sb, in_=xT_d[:, :].rearrange("(c p) t -> p c t", p=128))
            nc.sync.dma_start(out=wT_sb, in_=wT_d[:, :].rearrange("(c p) t -> p c t", p=128))
            nc.sync.dma_start(out=woT_sb, in_=woT_d[:, :].rearrange("(c p) t -> p c t", p=128))
            nc.sync.dma_start(out=ot_sb, in_=ot_d[:, :])
            nc.sync.dma_start(out=m2_sb, in_=m2_d[:, :])
            nc.vector.memset(ones_p, 1.0)
            v_ones = v_sb[:, :, :].rearrange("p a (h e) -> p a h e", e=D + 1)[:, :, :, D:D + 1]
            nc.vector.memset(v_ones, 1.0)
            # q column LIVE is pinned to 0 so exp gives p'=1 there: the pv
            # matmul's column LIVE-512 then lands [sum(v) | 1024] = the
            # masked-query numerator and denominator, with a single
            # start=True writer chain per PSUM bank.
            nc.vector.memset(qT_sb[:, :, LIVE:LIVE + 1], 0.0)

            # ---- Phase 1: v projection (natural layout, feeds all heads)
            with tc.tile_pool(name="v_ps", bufs=2, space="PSUM") as v_pool:
                for tt in range(8 if upto != "dma" else 0):
                    ps = v_pool.tile([128, GD], F32, tag="vps")
                    for ek in range(6):
                        nc.tensor.matmul(ps,
                                         xT_sb[:, ek, ts(tt, 128)],
                                         wT_sb[:, ek, 2 * GD:3 * GD],
                                         start=(ek == 0), stop=False)
                    nc.tensor.matmul(ps, ot_sb[:, ts(tt, 128)],
                                     m2_sb[:, 2 * GD:3 * GD], start=False, stop=True)
                    dst = v_sb[:, tt, :].rearrange("p (h e) -> p h e", e=D + 1)[:, :, 0:D]
                    src = ps[:, :].rearrange("p (h d) -> p h d", d=D)
                    nc.scalar.copy(dst, src)

            # ---- Phase 2: per head-pair: project q,k chunk then attend both
            # heads. Keeps PE dense (projection of pair c+1 overlaps the
            # ACT-bound softmax of pair c) so HAM stays warm.
            with tc.tile_pool(name="qk_ps", bufs=1, space="PSUM") as qk_pool, \
                 tc.tile_pool(name="sT_ps", bufs=2, space="PSUM") as sT_pool, \
                 tc.tile_pool(name="acc_ps", bufs=1, space="PSUM") as acc_pool, \
                 tc.tile_pool(name="acc2_ps", bufs=1, space="PSUM") as acc2_pool, \
                 tc.tile_pool(name="pp", bufs=3) as pp_pool, \
                 tc.tile_pool(name="sm", bufs=3) as sm_pool, \
                 tc.tile_pool(name="dscr", bufs=3, space="DRAM") as dr_pool:
                for c in range(3 if upto not in ("dma", "v") else 0):
                    for mt in (c, c + 3):    # q chunk then k chunk
                        ps = qk_pool.tile([128, T], F32, tag="qkps")
                        isq = mt < 3
                        slices = ((0, 512), (512, LIVE)) if isq else ((0, 512), (512, T))
                        for ek in range(6):
                            for s0, s1 in slices:
                                nc.tensor.matmul(ps[:, s0:s1],
                                                 wT_sb[:, ek, ts(mt, 128)],
                                                 xT_sb[:, ek, s0:s1],
                                                 start=(ek == 0), stop=False)
                        for s0, s1 in slices:
                            nc.tensor.matmul(ps[:, s0:s1],
                                             m2_sb[:, ts(mt, 128)],
                                             ot_sb[:, s0:s1],
                                             start=False, stop=True)
                        if isq:
                            nc.vector.tensor_copy(qT_sb[:, mt, 0:LIVE], ps[:, 0:LIVE])
                        else:
                            nc.vector.tensor_copy(kT_sb[:, mt - 3, :], ps[:, :])

                    for h in (2 * c, 2 * c + 1):
                        po = (h % 2) * 64
                        qh = qT_sb[po:po + 64, c, :]
                        kh = kT_sb[po:po + 64, c, :]
                        acc1 = acc_pool.tile([65, 512], F32, tag="acc1")
                        acc2 = acc2_pool.tile([65, 512], F32, tag="acc2")
                        for kt in range(8):
                            # [0:512) in bank 0, [512:673) in bank 1, aligned
                            sT = sT_pool.tile([128, T], F32, tag="sT")
                            pp = pp_pool.tile([128, LIVE + 1], BF16, tag="pp")
                            nc.tensor.matmul(sT[:, 0:512], kh[:, ts(kt, 128)],
                                             qh[:, 0:512], start=True, stop=True)
                            nc.tensor.matmul(sT[:, 512:LIVE + 1], kh[:, ts(kt, 128)],
                                             qh[:, 512:LIVE + 1], start=True, stop=True)
                            nc.scalar.activation(pp[:, 0:LIVE + 1], sT[:, 0:LIVE + 1],
                                                 mybir.ActivationFunctionType.Exp,
                                                 scale=SCALE)
                            if debug and h == 0:
                                nc.sync.dma_start(out=dbg_p[:, kt, :], in_=pp[:, 0:LIVE])
                                sc = sm_pool.tile([128, LIVE], F32, tag="dbgsc")
                                nc.vector.tensor_copy(sc, sT[:, 0:LIVE])
                                nc.sync.dma_start(out=dbg_s[:, kt, :], in_=sc)
                            vh = v_sb[:, kt, h * (D + 1):(h + 1) * (D + 1)]
                            nc.tensor.matmul(acc1[:, :], vh, pp[:, 0:512],
                                             start=(kt == 0), stop=(kt == 7))
                            nc.tensor.matmul(acc2[:, 0:LIVE - 512 + 1], vh,
                                             pp[:, 512:LIVE + 1],
                                             start=(kt == 0), stop=(kt == 7))
                        # epilogue: normalize by denominators (row 64 of acc)
                        rd1 = sm_pool.tile([1, 512], F32, tag="rd1")
                        rd2 = sm_pool.tile([1, LIVE - 512], F32, tag="rd2")
                        rd3 = sm_pool.tile([1, 1], F32, tag="rd3")
                        nc.vector.reciprocal(rd1, acc1[64:65, :])
                        nc.vector.reciprocal(rd2, acc2[64:65, 0:LIVE - 512])
                        nc.vector.reciprocal(rd3, acc2[64:65, LIVE - 512:LIVE - 512 + 1])
                        rb1 = sm_pool.tile([64, 512], F32, tag="rb1")
                        rb2 = sm_pool.tile([64, LIVE - 512], F32, tag="rb2")
                        rb3 = sm_pool.tile([64, 1], F32, tag="rb3")
                        for rb, rd, w in ((rb1, rd1, 512), (rb2, rd2, LIVE - 512),
                                          (rb3, rd3, 1)):
                            # partition-broadcast via DRAM roundtrip (DMA can
                            # replicate from linear memory; SBUF-source
                            # zero-stride partition APs are not allowed)
                            dscr = dr_pool.tile([1, w], F32, tag=f"d{w}")
                            nc.sync.dma_start(out=dscr, in_=rd[0:1, :])
                            src = dscr[0:1, :]
                            bc = bass.AP(tensor=src.tensor, offset=src.offset,
                                         ap=[[0, 64]] + [list(a) for a in src.ap[1:]])
                            nc.sync.dma_start(out=rb, in_=bc)
                        ah = attnT_sb[po:po + 64, c, :]
                        nc.vector.tensor_mul(ah[:, 0:512], acc1[0:64, :], rb1)
                        nc.vector.tensor_mul(ah[:, 512:LIVE], acc2[0:64, 0:LIVE - 512], rb2)
                        mv = sm_pool.tile([64, 1], F32, tag="mv")
                        nc.vector.tensor_scalar_mul(mv, acc2[0:64, LIVE - 512:LIVE - 512 + 1], rb3)
                        nc.vector.tensor_scalar_mul(ah[:, LIVE:T], ones_p[0:64, :], mv)

            if debug:
                for nm, t, sh in (("dbg_q", qT_sb, [128, 3, T]),
                                  ("dbg_k", kT_sb, [128, 3, T]),
                                  ("dbg_v", v_sb, [128, 8, HG * (D + 1)]),
                                  ("dbg_a", attnT_sb, [128, 3, T])):
                    dd = nc.dram_tensor(nm, sh, BF16, kind="ExternalOutput")
                    nc.sync.dma_start(out=dd[:, :, :], in_=t[:, :, :])

            # ---- Phase 3: output projection (partial over this group's dims)
            with tc.tile_pool(name="o_ps", bufs=3, space="PSUM") as o_pool, \
                 tc.tile_pool(name="ob", bufs=3) as ob_pool:
                for tt in range(8 if upto == "full" else 0):
                    ps = o_pool.tile([128, E], F32, tag="ops")
                    for s0, s1 in ((0, 512), (512, E)):
                        for c3 in range(3):
                            nc.tensor.matmul(ps[:, s0:s1],
                                             attnT_sb[:, c3, ts(tt, 128)],
                                             woT_sb[:, c3, s0:s1],
                                             start=(c3 == 0), stop=(c3 == 2))
                    ob = ob_pool.tile([128, E], F32, tag="ob")
                    nc.vector.tensor_copy(ob, ps)
                    nc.sync.dma_start(out=out_d[ts(tt, 128), :], in_=ob)

    nc.finalize()
    return nc


def _get_bass():
    global _nc
    if _nc is None:
        _nc = _build_bass()
    return _nc


def kernel(x, idx, struct_embed, w_qkv, w_out, b_out):
    global _perm
    if _perm is None:
        _perm = _perm_live_first()
    perm = _perm

    x = np.asarray(x, dtype=np.float32)
    idx = np.asarray(idx)
    struct_embed = np.asarray(struct_embed, dtype=np.float32)
    w_qkv = np.asarray(w_qkv, dtype=np.float32)
    w_out = np.asarray(w_out, dtype=np.float32)
    b_out = np.asarray(b_out, dtype=np.float32)

    sid = ((idx == 1) * 1 + (idx == 2) * 2 + (idx == 3) * 3).astype(np.int64)  # [B,T]
    oh = (sid[:, :, None] == np.arange(4)[None, None, :]).astype(np.float32)  # [B,T,4]

    bf = ml_dtypes.bfloat16
    in_maps = []
    for core in range(8):
        b, g = core // 2, core % 2
        wg = np.concatenate([w_qkv[g * GD:(g + 1) * GD],
                             w_qkv[E + g * GD:E + (g + 1) * GD],
                             w_qkv[2 * E + g * GD:2 * E + (g + 1) * GD]], axis=0)  # [3GD, E]
        in_maps.append({
            "xT": np.ascontiguousarray(x[b].T[:, perm]).astype(bf),
            "wT": np.ascontiguousarray(wg.T).astype(bf),
            "ot": np.ascontiguousarray(oh[b].T[:, perm]).astype(bf),
            "m2": (struct_embed @ wg.T).astype(bf),
            "woT": np.ascontiguousarray(w_out[:, g * GD:(g + 1) * GD].T).astype(bf),
        })

    res = run_bass_kernel_spmd(_get_bass(), in_maps, core_ids=list(range(8)))

    inv = np.empty(T, dtype=np.int64)
    inv[perm] = np.arange(T)
    out = np.empty((B, T, E), dtype=np.float32)
    for b in range(B):
        acc = res.results[2 * b]["out"] + res.results[2 * b + 1]["out"]
        out[b] = acc[inv] + b_out[None, :]
    return out


# revision 22
# speedup vs baseline: 4.6591x; 1.2515x over previous
"""Trainium2 Bass kernel for nn_MultiHeadAttention_8074538516581.

Sharding: 8 cores = batch(4) x head-group(2 groups of 6 heads).
Each core computes, for its (b, g):
  qkv slice projection (bf16 matmuls, fp32 psum accum, struct-embed term
  folded in as a rank-4 matmul O @ (SE @ W^T)), per-head attention with the
  reference's exact semantics (q/k rounded to bf16, fixed-shift-free softmax
  -- the row-max subtraction cancels in the normalization, the [-30,30] logit
  clip and the 1e5/1e-10 guards are provably inactive here), and the partial
  output projection over its 384 head-dims.
Host sums the two head-group partials per batch and adds b_out.

Token permutation: queries with (t % 64) % 3 == 0 are zeroed by the
reference's load mask, making their attention output mean(v) per head.
Tokens are permuted live-first so the 672 live queries are contiguous:
scores/exp/pv run only on live columns; the 352 masked columns get the
per-head mean(v) via one N=1 matmul + broadcast.
"""
import numpy as np
import ml_dtypes

import concourse.bass as bass
import concourse.mybir as mybir
import concourse.tile as tile
from concourse import bacc
from concourse.bass import ts
from concourse.bass_utils import run_bass_kernel_spmd

B, T, E = 4, 1024, 768
H, D = 12, 64
HG = 6                  # heads per group
GD = HG * D             # 384 head-dims per group
BLOCK_M = 64
LIVE = 672              # tokens with (t % BLOCK_M) % 3 != 0
MASK = T - LIVE         # 352
SCALE = 1.0 / 8.0       # 1/sqrt(64)

BF16 = mybir.dt.bfloat16
F32 = mybir.dt.float32

_perm = None
_nc = None


def _perm_live_first():
    t = np.arange(T)
    m = (t % BLOCK_M) % 3 == 0
    return np.concatenate([t[~m], t[m]])


def _build_bass(debug=False, repeat=1, upto="full"):
    nc = bacc.Bacc()
    xT_d = nc.dram_tensor("xT", [E, T], BF16, kind="ExternalInput")
    wT_d = nc.dram_tensor("wT", [E, 3 * GD], BF16, kind="ExternalInput")
    ot_d = nc.dram_tensor("ot", [4, T], BF16, kind="ExternalInput")
    m2_d = nc.dram_tensor("m2", [4, 3 * GD], BF16, kind="ExternalInput")
    woT_d = nc.dram_tensor("woT", [GD, E], BF16, kind="ExternalInput")
    out_d = nc.dram_tensor("out", [T, E], F32, kind="ExternalOutput")

    dbg_p = None
    if debug:
        dbg_p = nc.dram_tensor("dbg_p", [128, 8, LIVE], BF16, kind="ExternalOutput")
        dbg_s = nc.dram_tensor("dbg_s", [128, 8, LIVE], F32, kind="ExternalOutput")

    from contextlib import ExitStack
    with tile.TileContext(nc) as tc, ExitStack() as rep_ctx:
        with tc.tile_pool(name="singles", bufs=1) as singles:
            xT_sb = singles.tile([128, 6, T], BF16)
            wT_sb = singles.tile([128, 6, 3 * GD], BF16)
            woT_sb = singles.tile([128, 3, E], BF16)
            ot_sb = singles.tile([4, T], BF16)
            m2_sb = singles.tile([4, 3 * GD], BF16)
            ones_p = singles.tile([128, MASK], BF16)
            qT_sb = singles.tile([128, 3, T], BF16)   # cols LIVE: garbage, never read
            kT_sb = singles.tile([128, 3, T], BF16)
            v_sb = singles.tile([128, 8, HG * (D + 1)], BF16)  # per-head v | ones col
            attnT_sb = singles.tile([128, 3, T], BF16)

            nc.sync.dma_start(out=xT_sb, in_=xT_d[:, :].rearrange("(c p) t -> p c t", p=128))
            nc.sync.dma_start(out=wT_sb, in_=wT_d[:, :].rearrange("(c p) t -> p c t", p=128))
            nc.sync.dma_start(out=woT_sb, in_=woT_d[:, :].rearrange("(c p) t -> p c t", p=128))
            nc.sync.dma_start(out=ot_sb, in_=ot_d[:, :])
            nc.sync.dma_start(out=m2_sb, in_=m2_d[:, :])
            nc.vector.memset(ones_p, 1.0)
            v_ones = v_sb[:, :, :].rearrange("p a (h e) -> p a h e", e=D + 1)[:, :, :, D:D + 1]
            nc.vector.memset(v_ones, 1.0)
            # q column LIVE is pinned to 0 so exp gives p'=1 there: the pv
            # matmul's column LIVE-512 then lands [sum(v) | 1024] = the
            # masked-query numerator and denominator, with a single
            # start=True writer chain per PSUM bank.
            nc.vector.memset(qT_sb[:, :, LIVE:LIVE + 1], 0.0)

            if repeat > 1:
                rep_ctx.enter_context(tc.For_i(0, repeat, 1))

            # ---- Phase 1: v projection (natural layout, feeds all heads)
            with tc.tile_pool(name="v_ps", bufs=2, space="PSUM") as v_pool:
                for tt in range(8 if upto != "dma" else 0):
                    ps = v_pool.tile([128, GD], F32, tag="vps")
                    for ek in range(6):
                        nc.tensor.matmul(ps,
                                         xT_sb[:, ek, ts(tt, 128)],
                                         wT_sb[:, ek, 2 * GD:3 * GD],
                                         start=(ek == 0), stop=False)
                    nc.tensor.matmul(ps, ot_sb[:, ts(tt, 128)],
                                     m2_sb[:, 2 * GD:3 * GD], start=False, stop=True)
                    dst = v_sb[:, tt, :].rearrange("p (h e) -> p h e", e=D + 1)[:, :, 0:D]
                    src = ps[:, :].rearrange("p (h d) -> p h d", d=D)
                    nc.scalar.copy(dst, src)

            # ---- Phase 2: per head-pair: project q,k chunk then attend both
            # heads. Keeps PE dense (projection of pair c+1 overlaps the
            # ACT-bound softmax of pair c) so HAM stays warm.
            with tc.tile_pool(name="qk_ps", bufs=1, space="PSUM") as qk_pool, \
                 tc.tile_pool(name="sT_ps", bufs=2, space="PSUM") as sT_pool, \
                 tc.tile_pool(name="acc_ps", bufs=1, space="PSUM") as acc_pool, \
                 tc.tile_pool(name="acc2_ps", bufs=1, space="PSUM") as acc2_pool, \
                 tc.tile_pool(name="pp", bufs=3) as pp_pool, \
                 tc.tile_pool(name="sm", bufs=3) as sm_pool, \
                 tc.tile_pool(name="dscr", bufs=3, space="DRAM") as dr_pool:
                for c in range(3 if upto not in ("dma", "v") else 0):
                    for mt in (c, c + 3):    # q chunk then k chunk
                        ps = qk_pool.tile([128, T], F32, tag="qkps")
                        isq = mt < 3
                        slices = ((0, 512), (512, LIVE)) if isq else ((0, 512), (512, T))
                        for ek in range(6):
                            for s0, s1 in slices:
                                nc.tensor.matmul(ps[:, s0:s1],
                                                 wT_sb[:, ek, ts(mt, 128)],
                                                 xT_sb[:, ek, s0:s1],
                                                 start=(ek == 0), stop=False)
                        for s0, s1 in slices:
                            nc.tensor.matmul(ps[:, s0:s1],
                                             m2_sb[:, ts(mt, 128)],
                                             ot_sb[:, s0:s1],
                                             start=False, stop=True)
                        if isq:
                            nc.vector.tensor_copy(qT_sb[:, mt, 0:LIVE], ps[:, 0:LIVE])
                        else:
                            nc.vector.tensor_copy(kT_sb[:, mt - 3, :], ps[:, :])

                    for h in (2 * c, 2 * c + 1):
                        po = (h % 2) * 64
                        qh = qT_sb[po:po + 64, c, :]
                        kh = kT_sb[po:po + 64, c, :]
                        acc1 = acc_pool.tile([65, 512], F32, tag="acc1")
                        acc2 = acc2_pool.tile([65, 512], F32, tag="acc2")
                        for kt in range(8):
                            # [0:512) in bank 0, [512:673) in bank 1, aligned
                            sT = sT_pool.tile([128, T], F32, tag="sT")
                            pp = pp_pool.tile([128, LIVE + 1], BF16, tag="pp")
                            nc.tensor.matmul(sT[:, 0:512], kh[:, ts(kt, 128)],
                                             qh[:, 0:512], start=True, stop=True)
                            nc.tensor.matmul(sT[:, 512:LIVE + 1], kh[:, ts(kt, 128)],
                                             qh[:, 512:LIVE + 1], start=True, stop=True)
                            nc.scalar.activation(pp[:, 0:LIVE + 1], sT[:, 0:LIVE + 1],
                                                 mybir.ActivationFunctionType.Exp,
                                                 scale=SCALE)
                            if debug and h == 0:
                                nc.sync.dma_start(out=dbg_p[:, kt, :], in_=pp[:, 0:LIVE])
                                sc = sm_pool.tile([128, LIVE], F32, tag="dbgsc")
                                nc.vector.tensor_copy(sc, sT[:, 0:LIVE])
                                nc.sync.dma_start(out=dbg_s[:, kt, :], in_=sc)
                            vh = v_sb[:, kt, h * (D + 1):(h + 1) * (D + 1)]
                            nc.tensor.matmul(acc1[:, :], vh, pp[:, 0:512],
                                             start=(kt == 0), stop=(kt == 7))
                            nc.tensor.matmul(acc2[:, 0:LIVE - 512 + 1], vh,
                                             pp[:, 512:LIVE + 1],
                                             start=(kt == 0), stop=(kt == 7))
                        # epilogue: normalize by denominators (row 64 of acc)
                        rd1 = sm_pool.tile([1, 512], F32, tag="rd1")
                        rd2 = sm_pool.tile([1, LIVE - 512], F32, tag="rd2")
                        rd3 = sm_pool.tile([1, 1], F32, tag="rd3")
                        nc.vector.reciprocal(rd1, acc1[64:65, :])
                        nc.vector.reciprocal(rd2, acc2[64:65, 0:LIVE - 512])
                        nc.vector.reciprocal(rd3, acc2[64:65, LIVE - 512:LIVE - 512 + 1])
                        rb1 = sm_pool.tile([64, 512], F32, tag="rb1")
                        rb2 = sm_pool.tile([64, LIVE - 512], F32, tag="rb2")
                        rb3 = sm_pool.tile([64, 1], F32, tag="rb3")
                        for rb, rd, w in ((rb1, rd1, 512), (rb2, rd2, LIVE - 512),
                                          (rb3, rd3, 1)):
                            # partition-broadcast via DRAM roundtrip (DMA can
                            # replicate from linear memory; SBUF-source
                            # zero-stride partition APs are not allowed)
                            dscr = dr_pool.tile([1, w], F32, tag=f"d{w}")
                            nc.sync.dma_start(out=dscr, in_=rd[0:1, :])
                            src = dscr[0:1, :]
                            bc = bass.AP(tensor=src.tensor, offset=src.offset,
                                         ap=[[0, 64]] + [list(a) for a in src.ap[1:]])
                            nc.sync.dma_start(out=rb, in_=bc)
                        ah = attnT_sb[po:po + 64, c, :]
                        nc.vector.tensor_mul(ah[:, 0:512], acc1[0:64, :], rb1)
                        nc.vector.tensor_mul(ah[:, 512:LIVE], acc2[0:64, 0:LIVE - 512], rb2)
                        mv = sm_pool.tile([64, 1], F32, tag="mv")
                        nc.vector.tensor_scalar_mul(mv, acc2[0:64, LIVE - 512:LIVE - 512 + 1], rb3)
                        nc.vector.tensor_scalar_mul(ah[:, LIVE:T], ones_p[0:64, :], mv)

            if debug:
                for nm, t, sh in (("dbg_q", qT_sb, [128, 3, T]),
                                  ("dbg_k", kT_sb, [128, 3, T]),
                                  ("dbg_v", v_sb, [128, 8, HG * (D + 1)]),
                                  ("dbg_a", attnT_sb, [128, 3, T])):
                    dd = nc.dram_tensor(nm, sh, BF16, kind="ExternalOutput")
                    nc.sync.dma_start(out=dd[:, :, :], in_=t[:, :, :])

            # ---- Phase 3: output projection (partial over this group's dims)
            with tc.tile_pool(name="o_ps", bufs=3, space="PSUM") as o_pool, \
                 tc.tile_pool(name="ob", bufs=3) as ob_pool:
                for tt in range(8 if upto == "full" else 0):
                    ps = o_pool.tile([128, E], F32, tag="ops")
                    for s0, s1 in ((0, 512), (512, E)):
                        for c3 in range(3):
                            nc.tensor.matmul(ps[:, s0:s1],
                                             attnT_sb[:, c3, ts(tt, 128)],
                                             woT_sb[:, c3, s0:s1],
                                             start=(c3 == 0), stop=(c3 == 2))
                    ob = ob_pool.tile([128, E], F32, tag="ob")
                    nc.vector.tensor_copy(ob, ps)
                    nc.sync.dma_start(out=out_d[ts(tt, 128), :], in_=ob)

    nc.finalize()
    return nc


def _get_bass():
    global _nc
    if _nc is None:
        _nc = _build_bass()
    return _nc


def kernel(x, idx, struct_embed, w_qkv, w_out, b_out):
    global _perm
    if _perm is None:
        _perm = _perm_live_first()
    perm = _perm

    x = np.asarray(x, dtype=np.float32)
    idx = np.asarray(idx)
    struct_embed = np.asarray(struct_embed, dtype=np.float32)
    w_qkv = np.asarray(w_qkv, dtype=np.float32)
    w_out = np.asarray(w_out, dtype=np.float32)
    b_out = np.asarray(b_out, dtype=np.float32)

    sid = ((idx == 1) * 1 + (idx == 2) * 2 + (idx == 3) * 3).astype(np.int64)  # [B,T]
    oh = (sid[:, :, None] == np.arange(4)[None, None, :]).astype(np.float32)  # [B,T,4]

    bf = ml_dtypes.bfloat16
    in_maps = []
    for core in range(8):
        b, g = core // 2, core % 2
        wg = np.concatenate([w_qkv[g * GD:(g + 1) * GD],
                             w_qkv[E + g * GD:E + (g + 1) * GD],
                             w_qkv[2 * E + g * GD:2 * E + (g + 1) * GD]], axis=0)  # [3GD, E]
        in_maps.append({
            "xT": np.ascontiguousarray(x[b].T[:, perm]).astype(bf),
            "wT": np.ascontiguousarray(wg.T).astype(bf),
            "ot": np.ascontiguousarray(oh[b].T[:, perm]).astype(bf),
            "m2": (struct_embed @ wg.T).astype(bf),
            "woT": np.ascontiguousarray(w_out[:, g * GD:(g + 1) * GD].T).astype(bf),
        })

    res = run_bass_kernel_spmd(_get_bass(), in_maps, core_ids=list(range(8)))

    inv = np.empty(T, dtype=np.int64)
    inv[perm] = np.arange(T)
    out = np.empty((B, T, E), dtype=np.float32)
    for b in range(B):
        acc = res.results[2 * b]["out"] + res.results[2 * b + 1]["out"]
        out[b] = acc[inv] + b_out[None, :]
    return out


# revision 24
# speedup vs baseline: 7.0549x; 1.5142x over previous
"""Trainium2 Bass kernel for nn_MultiHeadAttention_8074538516581.

Sharding: 8 cores = batch(4) x head-group(2 groups of 6 heads).
Each core computes, for its (b, g):
  qkv slice projection (bf16 matmuls, fp32 psum accum, struct-embed term
  folded in as a rank-4 matmul O @ (SE @ W^T)), per-head attention with the
  reference's exact semantics (q/k rounded to bf16, fixed-shift-free softmax
  -- the row-max subtraction cancels in the normalization, the [-30,30] logit
  clip and the 1e5/1e-10 guards are provably inactive here), and the partial
  output projection over its 384 head-dims.
Host sums the two head-group partials per batch and adds b_out.

Token permutation: queries with (t % 64) % 3 == 0 are zeroed by the
reference's load mask, making their attention output mean(v) per head.
Tokens are permuted live-first so the 672 live queries are contiguous:
scores/exp/pv run only on live columns; the 352 masked columns get the
per-head mean(v) via one N=1 matmul + broadcast.
"""
import numpy as np
import ml_dtypes

import concourse.bass as bass
import concourse.mybir as mybir
import concourse.tile as tile
from concourse import bacc
from concourse.bass import ts
from concourse.bass_utils import run_bass_kernel_spmd

B, T, E = 4, 1024, 768
H, D = 12, 64
HG = 6                  # heads per group
GD = HG * D             # 384 head-dims per group
BLOCK_M = 64
LIVE = 672              # tokens with (t % BLOCK_M) % 3 != 0
MASK = T - LIVE         # 352
SCALE = 1.0 / 8.0       # 1/sqrt(64)

BF16 = mybir.dt.bfloat16
F32 = mybir.dt.float32

_perm = None
_nc = None


def _perm_live_first():
    t = np.arange(T)
    m = (t % BLOCK_M) % 3 == 0
    return np.concatenate([t[~m], t[m]])


def _build_bass(debug=False, repeat=1, upto="full"):
    nc = bacc.Bacc()
    xT_d = nc.dram_tensor("xT", [E, T], BF16, kind="ExternalInput")
    wT_d = nc.dram_tensor("wT", [E, 3 * GD], BF16, kind="ExternalInput")
    ot_d = nc.dram_tensor("ot", [4, T], BF16, kind="ExternalInput")
    m2_d = nc.dram_tensor("m2", [4, 3 * GD], BF16, kind="ExternalInput")
    woT_d = nc.dram_tensor("woT", [GD, E], BF16, kind="ExternalInput")
    out_d = nc.dram_tensor("out", [T, E], F32, kind="ExternalOutput")

    dbg_p = None
    if debug:
        dbg_p = nc.dram_tensor("dbg_p", [128, 8, LIVE], BF16, kind="ExternalOutput")
        dbg_s = nc.dram_tensor("dbg_s", [128, 8, LIVE], F32, kind="ExternalOutput")

    from contextlib import ExitStack
    with tile.TileContext(nc) as tc, ExitStack() as rep_ctx:
        with tc.tile_pool(name="singles", bufs=1) as singles:
            xT_sb = singles.tile([128, 6, T], BF16)
            wT_sb = singles.tile([128, 6, 3 * GD], BF16)
            woT_sb = singles.tile([128, 3, E], BF16)
            ot_sb = singles.tile([4, T], BF16)
            m2_sb = singles.tile([4, 3 * GD], BF16)
            ones_p = singles.tile([128, MASK], BF16)
            qT_sb = singles.tile([128, 3, T], BF16)   # cols LIVE: garbage, never read
            kT_sb = singles.tile([128, 3, T], BF16)
            v_sb = singles.tile([128, 8, HG * (D + 1)], BF16)  # per-head v | ones col
            attnT_sb = singles.tile([128, 3, T], BF16)

            nc.sync.dma_start(out=xT_sb, in_=xT_d[:, :].rearrange("(c p) t -> p c t", p=128))
            nc.sync.dma_start(out=wT_sb, in_=wT_d[:, :].rearrange("(c p) t -> p c t", p=128))
            nc.sync.dma_start(out=woT_sb, in_=woT_d[:, :].rearrange("(c p) t -> p c t", p=128))
            nc.sync.dma_start(out=ot_sb, in_=ot_d[:, :])
            nc.sync.dma_start(out=m2_sb, in_=m2_d[:, :])
            nc.vector.memset(ones_p, 1.0)
            v_ones = v_sb[:, :, :].rearrange("p a (h e) -> p a h e", e=D + 1)[:, :, :, D:D + 1]
            nc.vector.memset(v_ones, 1.0)
            # q column LIVE is pinned to 0 so exp gives p'=1 there: the pv
            # matmul's column LIVE-512 then lands [sum(v) | 1024] = the
            # masked-query numerator and denominator, with a single
            # start=True writer chain per PSUM bank.
            nc.vector.memset(qT_sb[:, :, LIVE:LIVE + 1], 0.0)

            if repeat > 1:
                rep_ctx.enter_context(tc.For_i(0, repeat, 1))

            # ---- Phase 1: v projection (natural layout, feeds all heads)
            with tc.tile_pool(name="v_ps", bufs=2, space="PSUM") as v_pool:
                for tt in range(8 if upto != "dma" else 0):
                    ps = v_pool.tile([128, GD], F32, tag="vps")
                    for ek in range(6):
                        nc.tensor.matmul(ps,
                                         xT_sb[:, ek, ts(tt, 128)],
                                         wT_sb[:, ek, 2 * GD:3 * GD],
                                         start=(ek == 0), stop=False)
                    nc.tensor.matmul(ps, ot_sb[:, ts(tt, 128)],
                                     m2_sb[:, 2 * GD:3 * GD], start=False, stop=True)
                    dst = v_sb[:, tt, :].rearrange("p (h e) -> p h e", e=D + 1)[:, :, 0:D]
                    src = ps[:, :].rearrange("p (h d) -> p h d", d=D)
                    nc.scalar.copy(dst, src)

            # ---- Phase 2: per head-pair: project q,k chunk then attend both
            # heads. Keeps PE dense (projection of pair c+1 overlaps the
            # ACT-bound softmax of pair c) so HAM stays warm.
            with tc.tile_pool(name="qk_ps", bufs=1, space="PSUM") as qk_pool, \
                 tc.tile_pool(name="sT_ps", bufs=2, space="PSUM") as sT_pool, \
                 tc.tile_pool(name="acc_ps", bufs=1, space="PSUM") as acc_pool, \
                 tc.tile_pool(name="acc2_ps", bufs=1, space="PSUM") as acc2_pool, \
                 tc.tile_pool(name="pp", bufs=3) as pp_pool, \
                 tc.tile_pool(name="sm", bufs=3) as sm_pool, \
                 tc.tile_pool(name="dscr", bufs=3, space="DRAM") as dr_pool:
                for c in range(3 if upto not in ("dma", "v") else 0):
                    for mt in (c, c + 3):    # q chunk then k chunk
                        ps = qk_pool.tile([128, T], F32, tag="qkps")
                        isq = mt < 3
                        slices = ((0, 512), (512, LIVE)) if isq else ((0, 512), (512, T))
                        for ek in range(6):
                            for s0, s1 in slices:
                                nc.tensor.matmul(ps[:, s0:s1],
                                                 wT_sb[:, ek, ts(mt, 128)],
                                                 xT_sb[:, ek, s0:s1],
                                                 start=(ek == 0), stop=False)
                        for s0, s1 in slices:
                            nc.tensor.matmul(ps[:, s0:s1],
                                             m2_sb[:, ts(mt, 128)],
                                             ot_sb[:, s0:s1],
                                             start=False, stop=True)
                        if isq:
                            nc.vector.tensor_copy(qT_sb[:, mt, 0:LIVE], ps[:, 0:LIVE])
                        else:
                            nc.vector.tensor_copy(kT_sb[:, mt - 3, :], ps[:, :])

                    for h in (2 * c, 2 * c + 1):
                        po = (h % 2) * 64
                        qh = qT_sb[po:po + 64, c, :]
                        kh = kT_sb[po:po + 64, c, :]
                        acc1 = acc_pool.tile([65, 512], F32, tag="acc1")
                        acc2 = acc2_pool.tile([65, 512], F32, tag="acc2")
                        for kt in range(8):
                            # [0:512) in bank 0, [512:673) in bank 1, aligned
                            sT = sT_pool.tile([128, T], F32, tag="sT")
                            pp = pp_pool.tile([128, LIVE + 1], BF16, tag="pp")
                            nc.tensor.matmul(sT[:, 0:512], kh[:, ts(kt, 128)],
                                             qh[:, 0:512], start=True, stop=True)
                            nc.tensor.matmul(sT[:, 512:LIVE + 1], kh[:, ts(kt, 128)],
                                             qh[:, 512:LIVE + 1], start=True, stop=True)
                            nc.scalar.activation(pp[:, 0:LIVE + 1], sT[:, 0:LIVE + 1],
                                                 mybir.ActivationFunctionType.Exp,
                                                 scale=SCALE)
                            if debug and h == 0:
                                nc.sync.dma_start(out=dbg_p[:, kt, :], in_=pp[:, 0:LIVE])
                                sc = sm_pool.tile([128, LIVE], F32, tag="dbgsc")
                                nc.vector.tensor_copy(sc, sT[:, 0:LIVE])
                                nc.sync.dma_start(out=dbg_s[:, kt, :], in_=sc)
                            vh = v_sb[:, kt, h * (D + 1):(h + 1) * (D + 1)]
                            nc.tensor.matmul(acc1[:, :], vh, pp[:, 0:512],
                                             start=(kt == 0), stop=(kt == 7))
                            nc.tensor.matmul(acc2[:, 0:LIVE - 512 + 1], vh,
                                             pp[:, 512:LIVE + 1],
                                             start=(kt == 0), stop=(kt == 7))
                        # epilogue: normalize by denominators (row 64 of acc)
                        rd1 = sm_pool.tile([1, 512], F32, tag="rd1")
                        rd2 = sm_pool.tile([1, LIVE - 512], F32, tag="rd2")
                        rd3 = sm_pool.tile([1, 1], F32, tag="rd3")
                        nc.vector.reciprocal(rd1, acc1[64:65, :])
                        nc.vector.reciprocal(rd2, acc2[64:65, 0:LIVE - 512])
                        nc.vector.reciprocal(rd3, acc2[64:65, LIVE - 512:LIVE - 512 + 1])
                        rb1 = sm_pool.tile([64, 512], F32, tag="rb1")
                        rb2 = sm_pool.tile([64, LIVE - 512], F32, tag="rb2")
                        rb3 = sm_pool.tile([64, 1], F32, tag="rb3")
                        for rb, rd, w in ((rb1, rd1, 512), (rb2, rd2, LIVE - 512),
                                          (rb3, rd3, 1)):
                            # partition-broadcast via DRAM roundtrip (DMA can
                            # replicate from linear memory; SBUF-source
                            # zero-stride partition APs are not allowed)
                            dscr = dr_pool.tile([1, w], F32, tag=f"d{w}")
                            nc.sync.dma_start(out=dscr, in_=rd[0:1, :])
                            src = dscr[0:1, :]
                            bc = bass.AP(tensor=src.tensor, offset=src.offset,
                                         ap=[[0, 64]] + [list(a) for a in src.ap[1:]])
                            nc.sync.dma_start(out=rb, in_=bc)
                        ah = attnT_sb[po:po + 64, c, :]
                        nc.vector.tensor_mul(ah[:, 0:512], acc1[0:64, :], rb1)
                        nc.vector.tensor_mul(ah[:, 512:LIVE], acc2[0:64, 0:LIVE - 512], rb2)
                        mv = sm_pool.tile([64, 1], F32, tag="mv")
                        nc.vector.tensor_scalar_mul(mv, acc2[0:64, LIVE - 512:LIVE - 512 + 1], rb3)
                        nc.vector.tensor_scalar_mul(ah[:, LIVE:T], ones_p[0:64, :], mv)

            if debug:
                for nm, t, sh in (("dbg_q", qT_sb, [128, 3, T]),
                                  ("dbg_k", kT_sb, [128, 3, T]),
                                  ("dbg_v", v_sb, [128, 8, HG * (D + 1)]),
                                  ("dbg_a", attnT_sb, [128, 3, T])):
                    dd = nc.dram_tensor(nm, sh, BF16, kind="ExternalOutput")
                    nc.sync.dma_start(out=dd[:, :, :], in_=t[:, :, :])

            # ---- Phase 3: output projection (partial over this group's dims)
            ob_singles = None
            if repeat > 1:
                ob_singles = []
                for i in range(8):
                    obs = singles.tile([128, E], F32, tag=f"obs{i}", name=f"obs{i}")
                    ob_singles.append(obs)
            with tc.tile_pool(name="o_ps", bufs=3, space="PSUM") as o_pool, \
                 tc.tile_pool(name="ob", bufs=3) as ob_pool:
                for tt in range(8 if upto == "full" else 0):
                    ps = o_pool.tile([128, E], F32, tag="ops")
                    for s0, s1 in ((0, 512), (512, E)):
                        for c3 in range(3):
                            nc.tensor.matmul(ps[:, s0:s1],
                                             attnT_sb[:, c3, ts(tt, 128)],
                                             woT_sb[:, c3, s0:s1],
                                             start=(c3 == 0), stop=(c3 == 2))
                    if repeat > 1:
                        nc.vector.tensor_copy(ob_singles[tt], ps)
                    else:
                        ob = ob_pool.tile([128, E], F32, tag="ob")
                        nc.vector.tensor_copy(ob, ps)
                        nc.sync.dma_start(out=out_d[ts(tt, 128), :], in_=ob)
            if repeat > 1:
                rep_ctx.close()
                for tt in range(8 if upto == "full" else 0):
                    nc.sync.dma_start(out=out_d[ts(tt, 128), :], in_=ob_singles[tt])

    nc.finalize()
    return nc


def _get_bass():
    global _nc
    if _nc is None:
        _nc = _build_bass()
    return _nc


def kernel(x, idx, struct_embed, w_qkv, w_out, b_out):
    global _perm
    if _perm is None:
        _perm = _perm_live_first()
    perm = _perm

    x = np.asarray(x, dtype=np.float32)
    idx = np.asarray(idx)
    struct_embed = np.asarray(struct_embed, dtype=np.float32)
    w_qkv = np.asarray(w_qkv, dtype=np.float32)
    w_out = np.asarray(w_out, dtype=np.float32)
    b_out = np.asarray(b_out, dtype=np.float32)

    sid = ((idx == 1) * 1 + (idx == 2) * 2 + (idx == 3) * 3).astype(np.int64)  # [B,T]
    oh = (sid[:, :, None] == np.arange(4)[None, None, :]).astype(np.float32)  # [B,T,4]

    bf = ml_dtypes.bfloat16
    in_maps = []
    for core in range(8):
        b, g = core // 2, core % 2
        wg = np.concatenate([w_qkv[g * GD:(g + 1) * GD],
                             w_qkv[E + g * GD:E + (g + 1) * GD],
                             w_qkv[2 * E + g * GD:2 * E + (g + 1) * GD]], axis=0)  # [3GD, E]
        in_maps.append({
            "xT": np.ascontiguousarray(x[b].T[:, perm]).astype(bf),
            "wT": np.ascontiguousarray(wg.T).astype(bf),
            "ot": np.ascontiguousarray(oh[b].T[:, perm]).astype(bf),
            "m2": (struct_embed @ wg.T).astype(bf),
            "woT": np.ascontiguousarray(w_out[:, g * GD:(g + 1) * GD].T).astype(bf),
        })

    res = run_bass_kernel_spmd(_get_bass(), in_maps, core_ids=list(range(8)))

    inv = np.empty(T, dtype=np.int64)
    inv[perm] = np.arange(T)
    out = np.empty((B, T, E), dtype=np.float32)
    for b in range(B):
        acc = res.results[2 * b]["out"] + res.results[2 * b + 1]["out"]
        out[b] = acc[inv] + b_out[None, :]
    return out


# revision 27
# speedup vs baseline: 9.2265x; 1.3078x over previous
"""Trainium2 Bass kernel for nn_MultiHeadAttention_8074538516581.

Sharding: 8 cores = batch(4) x head-group(2 groups of 6 heads).
Each core computes, for its (b, g):
  qkv slice projection (bf16 matmuls, fp32 psum accum, struct-embed term
  folded in as a rank-4 matmul O @ (SE @ W^T)), per-head attention with the
  reference's exact semantics (q/k rounded to bf16, fixed-shift-free softmax
  -- the row-max subtraction cancels in the normalization, the [-30,30] logit
  clip and the 1e5/1e-10 guards are provably inactive here), and the partial
  output projection over its 384 head-dims.
Host sums the two head-group partials per batch and adds b_out.

Token permutation: queries with (t % 64) % 3 == 0 are zeroed by the
reference's load mask, making their attention output mean(v) per head.
Tokens are permuted live-first so the 672 live queries are contiguous:
scores/exp/pv run only on live columns; the 352 masked columns get the
per-head mean(v) via one N=1 matmul + broadcast.
"""
import numpy as np
import ml_dtypes

import concourse.bass as bass
import concourse.mybir as mybir
import concourse.tile as tile
from concourse import bacc
from concourse.bass import ts
from concourse.bass_utils import run_bass_kernel_spmd

B, T, E = 4, 1024, 768
H, D = 12, 64
HG = 6                  # heads per group
GD = HG * D             # 384 head-dims per group
BLOCK_M = 64
LIVE = 672              # tokens with (t % BLOCK_M) % 3 != 0
MASK = T - LIVE         # 352
SCALE = 1.0 / 8.0       # 1/sqrt(64)

BF16 = mybir.dt.bfloat16
F32 = mybir.dt.float32

_perm = None
_nc = None


def _perm_live_first():
    t = np.arange(T)
    m = (t % BLOCK_M) % 3 == 0
    return np.concatenate([t[~m], t[m]])


def _build_bass(debug=False, repeat=1, upto="full"):
    nc = bacc.Bacc()
    xT_d = nc.dram_tensor("xT", [E, T], BF16, kind="ExternalInput")
    wT_d = nc.dram_tensor("wT", [E, 3 * GD], BF16, kind="ExternalInput")
    ot_d = nc.dram_tensor("ot", [4, T], BF16, kind="ExternalInput")
    m2_d = nc.dram_tensor("m2", [4, 3 * GD], BF16, kind="ExternalInput")
    woT_d = nc.dram_tensor("woT", [GD, E], BF16, kind="ExternalInput")
    out_d = nc.dram_tensor("out", [T, E], F32, kind="ExternalOutput")

    dbg_p = None
    if debug:
        dbg_p = nc.dram_tensor("dbg_p", [128, 8, LIVE], BF16, kind="ExternalOutput")
        dbg_s = nc.dram_tensor("dbg_s", [128, 8, LIVE], F32, kind="ExternalOutput")

    from contextlib import ExitStack
    with tile.TileContext(nc) as tc, ExitStack() as rep_ctx:
        with tc.tile_pool(name="singles", bufs=1) as singles:
            xT_sb = singles.tile([128, 6, T], BF16)
            wT_sb = singles.tile([128, 6, 3 * GD], BF16)
            woT_sb = singles.tile([128, 3, E], BF16)
            ot_sb = singles.tile([4, T], BF16)
            m2_sb = singles.tile([4, 3 * GD], BF16)
            ones_p = singles.tile([128, MASK], BF16)
            qT_sb = singles.tile([128, 3, T], BF16)   # cols LIVE: garbage, never read
            kT_sb = singles.tile([128, 3, T], BF16)
            v_sb = singles.tile([128, 8, HG * (D + 1)], BF16)  # per-head v | ones col
            attnT_sb = singles.tile([128, 3, T], BF16)

            nc.sync.dma_start(out=xT_sb, in_=xT_d[:, :].rearrange("(c p) t -> p c t", p=128))
            nc.sync.dma_start(out=wT_sb, in_=wT_d[:, :].rearrange("(c p) t -> p c t", p=128))
            nc.sync.dma_start(out=woT_sb, in_=woT_d[:, :].rearrange("(c p) t -> p c t", p=128))
            nc.sync.dma_start(out=ot_sb, in_=ot_d[:, :])
            nc.sync.dma_start(out=m2_sb, in_=m2_d[:, :])
            nc.vector.memset(ones_p, 1.0)
            v_ones = v_sb[:, :, :].rearrange("p a (h e) -> p a h e", e=D + 1)[:, :, :, D:D + 1]
            nc.vector.memset(v_ones, 1.0)
            # q column LIVE is pinned to 0 so exp gives p'=1 there: the pv
            # matmul's column LIVE-512 then lands [sum(v) | 1024] = the
            # masked-query numerator and denominator, with a single
            # start=True writer chain per PSUM bank.
            nc.vector.memset(qT_sb[:, :, LIVE:LIVE + 1], 0.0)

            if repeat > 1:
                rep_ctx.enter_context(tc.For_i(0, repeat, 1))

            # ---- Phase 1: v projection (natural layout, feeds all heads)
            with tc.tile_pool(name="v_ps", bufs=2, space="PSUM") as v_pool:
                for tt in range(8 if upto != "dma" else 0):
                    ps = v_pool.tile([128, GD], F32, tag="vps")
                    for ek in range(6):
                        nc.tensor.matmul(ps,
                                         xT_sb[:, ek, ts(tt, 128)],
                                         wT_sb[:, ek, 2 * GD:3 * GD],
                                         start=(ek == 0), stop=False)
                    nc.tensor.matmul(ps, ot_sb[:, ts(tt, 128)],
                                     m2_sb[:, 2 * GD:3 * GD], start=False, stop=True)
                    dst = v_sb[:, tt, :].rearrange("p (h e) -> p h e", e=D + 1)[:, :, 0:D]
                    src = ps[:, :].rearrange("p (h d) -> p h d", d=D)
                    nc.scalar.copy(dst, src)

            # ---- Phase 2: per head-pair: project q,k chunk then attend both
            # heads. Keeps PE dense (projection of pair c+1 overlaps the
            # ACT-bound softmax of pair c) so HAM stays warm.
            with tc.tile_pool(name="qk_ps", bufs=1, space="PSUM") as qk_pool, \
                 tc.tile_pool(name="sT_ps", bufs=2, space="PSUM") as sT_pool, \
                 tc.tile_pool(name="acc_ps", bufs=1, space="PSUM") as acc_pool, \
                 tc.tile_pool(name="pp", bufs=3) as pp_pool, \
                 tc.tile_pool(name="sm", bufs=3) as sm_pool, \
                 tc.tile_pool(name="dscr", bufs=3, space="DRAM") as dr_pool:
                for c in range(3 if upto not in ("dma", "v") else 0):
                    for mt in (c, c + 3):    # q chunk then k chunk
                        ps = qk_pool.tile([128, T], F32, tag="qkps")
                        isq = mt < 3
                        slices = ((0, 512), (512, LIVE)) if isq else ((0, 512), (512, T))
                        for ek in range(6):
                            for s0, s1 in slices:
                                nc.tensor.matmul(ps[:, s0:s1],
                                                 wT_sb[:, ek, ts(mt, 128)],
                                                 xT_sb[:, ek, s0:s1],
                                                 start=(ek == 0), stop=False)
                        for s0, s1 in slices:
                            nc.tensor.matmul(ps[:, s0:s1],
                                             m2_sb[:, ts(mt, 128)],
                                             ot_sb[:, s0:s1],
                                             start=False, stop=True)
                        if isq:
                            nc.vector.tensor_copy(qT_sb[:, mt, 0:LIVE], ps[:, 0:LIVE])
                        else:
                            nc.vector.tensor_copy(kT_sb[:, mt - 3, :], ps[:, :])

                    for h in (2 * c, 2 * c + 1):
                        po = (h % 2) * 64
                        qh = qT_sb[po:po + 64, c, :]
                        kh = kT_sb[po:po + 64, c, :]
                        acc1 = acc_pool.tile([65, T], F32, tag="acc1")
                        for kt in range(8):
                            # [0:512) in bank 0, [512:673) in bank 1, aligned
                            sT = sT_pool.tile([128, T], F32, tag="sT")
                            pp = pp_pool.tile([128, LIVE + 1], BF16, tag="pp")
                            nc.tensor.matmul(sT[:, 0:512], kh[:, ts(kt, 128)],
                                             qh[:, 0:512], start=True, stop=True)
                            nc.tensor.matmul(sT[:, 512:LIVE + 1], kh[:, ts(kt, 128)],
                                             qh[:, 512:LIVE + 1], start=True, stop=True)
                            nc.scalar.activation(pp[:, 0:LIVE + 1], sT[:, 0:LIVE + 1],
                                                 mybir.ActivationFunctionType.Exp,
                                                 scale=SCALE)
                            if debug and h == 0:
                                nc.sync.dma_start(out=dbg_p[:, kt, :], in_=pp[:, 0:LIVE])
                                sc = sm_pool.tile([128, LIVE], F32, tag="dbgsc")
                                nc.vector.tensor_copy(sc, sT[:, 0:LIVE])
                                nc.sync.dma_start(out=dbg_s[:, kt, :], in_=sc)
                            vh = v_sb[:, kt, h * (D + 1):(h + 1) * (D + 1)]
                            nc.tensor.matmul(acc1[:, 0:512], vh, pp[:, 0:512],
                                             start=(kt == 0), stop=(kt == 7))
                            nc.tensor.matmul(acc1[:, 512:LIVE + 1], vh,
                                             pp[:, 512:LIVE + 1],
                                             start=(kt == 0), stop=(kt == 7))
                        # copy acc out of PSUM right away so the next head's
                        # pv matmuls don't wait on the whole epilogue chain
                        acc_sb = sm_pool.tile([65, LIVE + 1], F32, tag="accsb")
                        nc.vector.tensor_copy(acc_sb, acc1[0:65, 0:LIVE + 1])
                        # normalize by denominators (row 64)
                        rd = sm_pool.tile([1, LIVE + 1], F32, tag="rd")
                        nc.vector.reciprocal(rd, acc_sb[64:65, :])
                        # partition-broadcast via DRAM roundtrip (DMA can
                        # replicate from linear memory; SBUF-source
                        # zero-stride partition APs are not allowed)
                        rb = sm_pool.tile([64, LIVE + 1], F32, tag="rb")
                        dscr = dr_pool.tile([1, LIVE + 1], F32, tag="dscr")
                        nc.sync.dma_start(out=dscr, in_=rd[0:1, :])
                        src = dscr[0:1, :]
                        bc = bass.AP(tensor=src.tensor, offset=src.offset,
                                     ap=[[0, 64]] + [list(a) for a in src.ap[1:]])
                        nc.sync.dma_start(out=rb, in_=bc)
                        ah = attnT_sb[po:po + 64, c, :]
                        nc.vector.tensor_mul(ah[:, 0:LIVE], acc_sb[0:64, 0:LIVE],
                                             rb[:, 0:LIVE])
                        mv = sm_pool.tile([64, 1], F32, tag="mv")
                        nc.vector.tensor_scalar_mul(mv, acc_sb[0:64, LIVE:LIVE + 1],
                                                    rb[0:64, LIVE:LIVE + 1])
                        nc.vector.tensor_scalar_mul(ah[:, LIVE:T], ones_p[0:64, :], mv)

            if debug:
                for nm, t, sh in (("dbg_q", qT_sb, [128, 3, T]),
                                  ("dbg_k", kT_sb, [128, 3, T]),
                                  ("dbg_v", v_sb, [128, 8, HG * (D + 1)]),
                                  ("dbg_a", attnT_sb, [128, 3, T])):
                    dd = nc.dram_tensor(nm, sh, BF16, kind="ExternalOutput")
                    nc.sync.dma_start(out=dd[:, :, :], in_=t[:, :, :])

            # ---- Phase 3: output projection (partial over this group's dims)
            ob_singles = None
            if repeat > 1:
                ob_singles = []
                for i in range(8):
                    obs = singles.tile([128, E], F32, tag=f"obs{i}", name=f"obs{i}")
                    ob_singles.append(obs)
            with tc.tile_pool(name="o_ps", bufs=3, space="PSUM") as o_pool, \
                 tc.tile_pool(name="ob", bufs=3) as ob_pool:
                for tt in range(8 if upto == "full" else 0):
                    ps = o_pool.tile([128, E], F32, tag="ops")
                    for s0, s1 in ((0, 512), (512, E)):
                        for c3 in range(3):
                            nc.tensor.matmul(ps[:, s0:s1],
                                             attnT_sb[:, c3, ts(tt, 128)],
                                             woT_sb[:, c3, s0:s1],
                                             start=(c3 == 0), stop=(c3 == 2))
                    if repeat > 1:
                        nc.vector.tensor_copy(ob_singles[tt], ps)
                    else:
                        ob = ob_pool.tile([128, E], F32, tag="ob")
                        nc.vector.tensor_copy(ob, ps)
                        nc.sync.dma_start(out=out_d[ts(tt, 128), :], in_=ob)
            if repeat > 1:
                rep_ctx.close()
                for tt in range(8 if upto == "full" else 0):
                    nc.sync.dma_start(out=out_d[ts(tt, 128), :], in_=ob_singles[tt])

    nc.finalize()
    return nc


def _get_bass():
    global _nc
    if _nc is None:
        _nc = _build_bass()
    return _nc


def kernel(x, idx, struct_embed, w_qkv, w_out, b_out):
    global _perm
    if _perm is None:
        _perm = _perm_live_first()
    perm = _perm

    x = np.asarray(x, dtype=np.float32)
    idx = np.asarray(idx)
    struct_embed = np.asarray(struct_embed, dtype=np.float32)
    w_qkv = np.asarray(w_qkv, dtype=np.float32)
    w_out = np.asarray(w_out, dtype=np.float32)
    b_out = np.asarray(b_out, dtype=np.float32)

    sid = ((idx == 1) * 1 + (idx == 2) * 2 + (idx == 3) * 3).astype(np.int64)  # [B,T]
    oh = (sid[:, :, None] == np.arange(4)[None, None, :]).astype(np.float32)  # [B,T,4]

    bf = ml_dtypes.bfloat16
    in_maps = []
    for core in range(8):
        b, g = core // 2, core % 2
        wg = np.concatenate([w_qkv[g * GD:(g + 1) * GD],
                             w_qkv[E + g * GD:E + (g + 1) * GD],
                             w_qkv[2 * E + g * GD:2 * E + (g + 1) * GD]], axis=0)  # [3GD, E]
        in_maps.append({
            "xT": np.ascontiguousarray(x[b].T[:, perm]).astype(bf),
            "wT": np.ascontiguousarray(wg.T).astype(bf),
            "ot": np.ascontiguousarray(oh[b].T[:, perm]).astype(bf),
            "m2": (struct_embed @ wg.T).astype(bf),
            "woT": np.ascontiguousarray(w_out[:, g * GD:(g + 1) * GD].T).astype(bf),
        })

    res = run_bass_kernel_spmd(_get_bass(), in_maps, core_ids=list(range(8)))

    inv = np.empty(T, dtype=np.int64)
    inv[perm] = np.arange(T)
    out = np.empty((B, T, E), dtype=np.float32)
    for b in range(B):
        acc = res.results[2 * b]["out"] + res.results[2 * b + 1]["out"]
        out[b] = acc[inv] + b_out[None, :]
    return out
